# revision 32
# baseline (speedup 1.0000x reference)
"""Trainium2 Bass kernel for nn_CCPL_14216341750304 (CCPL / PatchNCE loss).

Math (per batch b, one per NeuronCore, 8 cores):
    Z_m = f_m[b].T @ W0  for m in {q, k}            # [HW, 64], fused into PE transpose
    g_c = Z[c_ids], g_n = Z[n_ids]                  # runtime gather
    H   = relu(g_c - g_n + b0); E = H @ W1 + b1     # MLP
    F   = E / (||E||_2 + eps)                       # L2 norm over 16 ch
    M   = Fq.T @ Fk   [S, S]                        # cosine sims, |M| <= 1
    loss_row s = 1/tau + log(sum_t exp((M[s,t]-1)/tau)) - M[s,s]/tau
l_pos is exactly diag(M); masking the diag with -inf and concatenating
l_pos yields the same logsumexp multiset as the unmasked row.  |M|<=1
lets a constant shift of 1 replace the row-max (no overflow, no masking).

Implementation per core:
  Phase T: stream f (q||k stacked on 128 partitions) from HBM; one PE
    matmul per 128 pixels: lhsT = f-chunk [128ch, 128px], rhs =
    blockdiag(W0, W0) -> PSUM [128px, 128ch] — transpose + W0 in one op.
    Pixels pair-interleaved so a pixel PAIR forms one contiguous 512B
    row [even: q64,k64 | odd: q64,k64] fp16 of a DRAM scratch
    [32768 pairs, 256].
  Gather: 16+16 indirect DMAs (stock InstDMACopy, one row per partition,
    idx = pair id int32) -> [128, 256] fp16 tiles; parity resolved with
    copy_predicated; diff; PE transpose back to [128ch, S].
  MLP/normalize on [128, S]/[16, S]; NCE via 16 M-chunks of [128, 2048]
  PSUM, exp+rowsum fused on ScalarE (accum_out).
  Output [1, 2] per core: [sum_s log(rowsum_s), sum_s l_pos_s].
Host: loss = sum_cores(S/tau + o0 - o1/tau) / (8*S).
"""

import numpy as np

import concourse.bacc as bacc
import concourse.bass as bass
import concourse.mybir as mybir
import concourse.tile as tile
from concourse import bass_utils
from concourse.bass import ds, ts

F32 = mybir.dt.float32
F16 = mybir.dt.float16
BF16 = mybir.dt.bfloat16
I32 = mybir.dt.int32
I8 = mybir.dt.int8

B, C, H, W = 8, 64, 256, 256
HW = H * W                 # 65536
S = 2048                   # samples per batch (8*256)
NJ = S // 128              # 16 gather blocks per id set
TAU = 0.07
EPS = 1e-7
NCORES = 8
CHUNK = 4096               # pixels per DMA load chunk
NCHUNK = HW // CHUNK       # 16
PSCHUNK = 2048             # pixels per PSUM tile (8 superblocks of 256)
NPT = HW // PSCHUNK        # 32 psum tiles
EXPBIAS = -1.0 / TAU       # exp((M-1)/tau) = exp(M*(1/tau) + (-1/tau))

_CACHE = {}


def _build(n_bodies=1, stop_after=None, loop_n=0):
    """Build + compile the per-core Bass program (cached).

    n_bodies > 1 emits the whole kernel body multiple times back-to-back —
    used by the test harness to difference out fixed call overheads.
    stop_after in {"load", "T", "gather", "mlp"} truncates the body (for
    phase attribution in perf experiments).
    loop_n > 0 wraps the body in a device-side For loop of that many
    iterations (perf amplification).
    """
    key = f"nc{n_bodies}_{stop_after}_{loop_n}"
    if key in _CACHE:
        return _CACHE[key]

    nc = bacc.Bacc("TRN2", target_bir_lowering=False, debug=False)

    def dram_in(name, shape, dt):
        return nc.dram_tensor(name, shape, dt, kind="ExternalInput").ap()

    fs_d = dram_in("fs", [128, HW], F32)        # rows 0-63 fq ch, 64-127 fk ch
    wblk_d = dram_in("wblk", [128, 128], BF16)  # blockdiag(W0, W0)
    w1q_d = dram_in("w1q", [128, 16], BF16)     # W1 rows 0-63, zeros below
    w1k_d = dram_in("w1k", [128, 16], BF16)     # zeros, W1 rows 64-127
    b0b_d = dram_in("b0b", [128, 1], F32)       # [b0; b0]
    b1c_d = dram_in("b1c", [16, 1], F32)
    ones16_d = dram_in("ones16", [16, 16], F32)
    ones128_d = dram_in("ones128", [128, 1], F32)
    ident_d = dram_in("ident", [128, 128], F16)  # fp16 identity for PE transpose
    idxc_d = dram_in("idxc", [128, NJ], I32)    # pair ids, w[p,j] = pid[j*128+p]
    idxn_d = dram_in("idxn", [128, NJ], I32)
    maskc_d = dram_in("maskc", [128, NJ], I8)   # odd-parity bit per sample
    maskn_d = dram_in("maskn", [128, NJ], I8)
    out_d = nc.dram_tensor("out", [1, 2], F32, kind="ExternalOutput").ap()
    scr_d = nc.dram_tensor("zscr", [HW // 2, 256], F16, kind="Internal").ap()

    AF = mybir.ActivationFunctionType

    with tile.TileContext(nc) as tc:
        if loop_n:
            with tc.For_i(0, loop_n, 1):
                _emit_body(nc, tc, 0, AF, fs_d, wblk_d, w1q_d, w1k_d,
                           b0b_d, b1c_d, ones16_d, ones128_d, ident_d,
                           idxc_d, idxn_d, maskc_d, maskn_d, out_d, scr_d,
                           stop_after)
        else:
            for _body_i in range(n_bodies):
                _emit_body(nc, tc, _body_i, AF, fs_d, wblk_d, w1q_d, w1k_d,
                           b0b_d, b1c_d, ones16_d, ones128_d, ident_d,
                           idxc_d, idxn_d, maskc_d, maskn_d, out_d, scr_d,
                           stop_after)

    nc.compile()
    _CACHE[key] = nc
    return nc


def _emit_body(nc, tc, uid, AF, fs_d, wblk_d, w1q_d, w1k_d, b0b_d, b1c_d,
               ones16_d, ones128_d, ident_d, idxc_d, idxn_d, maskc_d,
               maskn_d, out_d, scr_d, stop_after=None):
        with (
            tc.tile_pool(name=f"const{uid}", bufs=1) as cp,
            tc.tile_pool(name=f"work{uid}", bufs=1) as wp,
            tc.tile_pool(name=f"psum{uid}", bufs=2,
                         space=bass.MemorySpace.PSUM) as pp,
        ):
            # ---- constants ----
            wblk = cp.tile([128, 128], BF16)
            nc.sync.dma_start(wblk[:], wblk_d)
            w1q = cp.tile([128, 16], BF16)
            nc.sync.dma_start(w1q[:], w1q_d)
            w1k = cp.tile([128, 16], BF16)
            nc.sync.dma_start(w1k[:], w1k_d)
            b0b = cp.tile([128, 1], F32)
            nc.sync.dma_start(b0b[:], b0b_d)
            b1c = cp.tile([16, 1], F32)
            nc.sync.dma_start(b1c[:], b1c_d)
            ones16 = cp.tile([16, 16], F32)
            nc.sync.dma_start(ones16[:], ones16_d)
            ones128 = cp.tile([128, 1], F32)
            nc.sync.dma_start(ones128[:], ones128_d)
            ident = cp.tile([128, 128], F16)
            nc.sync.dma_start(ident[:], ident_d)
            idxc = cp.tile([128, NJ], I32)
            nc.sync.dma_start(idxc[:], idxc_d)
            idxn = cp.tile([128, NJ], I32)
            nc.sync.dma_start(idxn[:], idxn_d)
            maskc = cp.tile([128, NJ], I8)
            nc.sync.dma_start(maskc[:], maskc_d)
            maskn = cp.tile([128, NJ], I8)
            nc.sync.dma_start(maskn[:], maskn_d)
            ebias = cp.tile([128, 1], F32)
            nc.gpsimd.memset(ebias[:], EXPBIAS)

            # ---- Phase T: transpose+W0 -> fp16 pair-slot scratch in DRAM ----
            # scratch row layout: [even px: q0..63,k0..63 | odd px: ...] fp16
            # = 512B. Row order is PERMUTED so each staging partition's 8
            # ranks are contiguous in DRAM (4KB per descriptor instead of
            # 512B): row(pid) = (pid//1024)*1024 + (pid%128)*8 + (pid//128)%8.
            # The host folds this permutation into the gather indices.
            scr_v = scr_d.rearrange("(pt p rl) e -> pt p rl e", p=128, rl=8)
            with tc.tile_pool(name=f"fst{uid}", bufs=2) as fsp:
                for ci in range(NCHUNK):
                    # SWDGE cast-DMA: f32 HBM -> bf16 SBUF at line rate
                    fstg = fsp.tile([128, CHUNK], BF16, tag="fst")
                    nc.gpsimd.dma_start(fstg[:], fs_d[:, ts(ci, CHUNK)])
                    if stop_after == "load":
                        continue
                    # free idx = h*2048 + sb*256 + q*2 + t
                    fsr = fstg[:].rearrange(
                        "p (h sb q t) -> p h sb t q", h=CHUNK // PSCHUNK,
                        sb=8, t=2
                    )
                    for hh in range(CHUNK // PSCHUNK):
                        ps = pp.tile([128, PSCHUNK], F32, tag="ps")
                        for sb in range(8):
                            for t in range(2):
                                nc.tensor.matmul(
                                    out=ps[:, ds(sb * 256 + t * 128, 128)],
                                    lhsT=fsr[:, hh, sb, t, :],
                                    rhs=wblk[:],
                                    start=True,
                                    stop=True,
                                )
                        pt = ci * (CHUNK // PSCHUNK) + hh
                        stg = fsp.tile([128, PSCHUNK], F16, tag="stg")
                        if pt % 2 == 0:
                            nc.vector.tensor_copy(stg[:], ps[:])
                        else:
                            nc.scalar.copy(stg[:], ps[:])
                        nc.sync.dma_start(
                            scr_v[pt],
                            stg[:].rearrange("p (rl e) -> p rl e", rl=8),
                        )

            if stop_after in ("load", "T"):
                dummy = wp.tile([1, 2], F32)
                nc.gpsimd.memset(dummy[:], 0.0)
                nc.sync.dma_start(out_d, dummy[:])
                return

            # ---- gathers: 16 blocks x [128 rows, 256] per id set ----
            gcs, gns = [], []
            for j in range(NJ):
                g = wp.tile([128, 256], F16, tag=f"gc{j}")
                nc.gpsimd.indirect_dma_start(
                    out=g[:], out_offset=None, in_=scr_d,
                    in_offset=bass.IndirectOffsetOnAxis(ap=idxc[:, j:j + 1], axis=0),
                )
                gcs.append(g)
                g = wp.tile([128, 256], F16, tag=f"gn{j}")
                nc.gpsimd.indirect_dma_start(
                    out=g[:], out_offset=None, in_=scr_d,
                    in_offset=bass.IndirectOffsetOnAxis(ap=idxn[:, j:j + 1], axis=0),
                )
                gns.append(g)

            if stop_after == "gather":
                dummy = wp.tile([1, 2], F32)
                nc.vector.tensor_copy(dummy[:], gcs[0][0:1, 0:2])
                nc.sync.dma_start(out_d, dummy[:])
                return

            # ---- parity select + diff (s-rows orientation) ----
            djall = wp.tile([128, S], F16)
            for j in range(NJ):
                mc = maskc[:, j:j + 1].to_broadcast([128, 128])
                mn = maskn[:, j:j + 1].to_broadcast([128, 128])
                nc.vector.copy_predicated(gcs[j][:, 0:128], mc, gcs[j][:, 128:256])
                nc.vector.copy_predicated(gns[j][:, 0:128], mn, gns[j][:, 128:256])
                nc.vector.tensor_sub(
                    djall[:, ts(j, 128)], gcs[j][:, 0:128], gns[j][:, 0:128]
                )

            # ---- transpose diff blocks back to [128ch, S] ----
            hin = wp.tile([128, S], F16)
            for j in range(NJ):
                pst = pp.tile([128, 128], F16, tag="ps")
                nc.tensor.transpose(
                    out=pst[:], in_=djall[:, ts(j, 128)], identity=ident[:]
                )
                if j % 2 == 0:
                    nc.vector.tensor_copy(hin[:, ts(j, 128)], pst[:])
                else:
                    nc.scalar.copy(hin[:, ts(j, 128)], pst[:])

            hid = wp.tile([128, S], BF16)
            nc.scalar.activation(hid[:], hin[:], AF.Relu, bias=b0b[:, 0:1])

            # ---- MLP layer 2 + L2 normalize ----
            fn = []
            for mi, w1t in enumerate((w1q, w1k)):
                psE = pp.tile([16, S], F32, tag="ps")
                for j in range(4):
                    nc.tensor.matmul(
                        out=psE[:, ts(j, 512)],
                        lhsT=w1t[:],
                        rhs=hid[:, ts(j, 512)],
                        start=True,
                        stop=True,
                    )
                emb = wp.tile([16, S], F32, tag="emb")
                nc.scalar.activation(emb[:], psE[:], AF.Identity, bias=b1c[:, 0:1])
                sq = wp.tile([16, S], F32, tag="sq")
                nc.vector.tensor_mul(sq[:], emb[:], emb[:])
                psN = pp.tile([16, S], F32, tag="ps")
                for j in range(4):
                    nc.tensor.matmul(
                        out=psN[:, ts(j, 512)],
                        lhsT=ones16[:],
                        rhs=sq[:, ts(j, 512)],
                        start=True,
                        stop=True,
                    )
                nrm = wp.tile([16, S], F32, tag="nrm")
                nc.scalar.activation(nrm[:], psN[:], AF.Sqrt)
                nrme = wp.tile([16, S], F32, tag="nrme")
                nc.vector.tensor_scalar_add(nrme[:], nrm[:], EPS)
                inv = wp.tile([16, S], F32, tag="inv")
                nc.vector.reciprocal_approx_fast(inv[:], nrme[:])
                f_n = wp.tile([16, S], F32, tag=f"fn{mi}")
                nc.vector.tensor_mul(f_n[:], emb[:], inv[:])
                fn.append(f_n)
            fqn, fkn = fn

            out_sb = wp.tile([1, 2], F32)

            if stop_after == "mlp":
                nc.vector.tensor_copy(out_sb[:], fqn[0:1, 0:2])
                nc.sync.dma_start(out_d, out_sb[:])
                return

            # bf16 copies for the NCE matmuls (PE runs bf16 at full rate)
            fqb = wp.tile([16, S], BF16)
            nc.vector.tensor_copy(fqb[:], fqn[:])
            fkb = wp.tile([16, S], BF16)
            nc.scalar.copy(fkb[:], fkn[:])

            # ---- l_pos = diag(M) = sum_c fqn*fkn ----
            pprod = wp.tile([16, S], F32, tag="sq")
            nc.vector.tensor_mul(pprod[:], fqn[:], fkn[:])
            psL = pp.tile([1, S], F32, tag="ps")
            for j in range(4):
                nc.tensor.matmul(
                    out=psL[:, ts(j, 512)],
                    lhsT=ones16[:, 0:1],
                    rhs=pprod[:, ts(j, 512)],
                    start=True,
                    stop=True,
                )
            nc.vector.tensor_reduce(
                out_sb[0:1, 1:2], psL[:], axis=mybir.AxisListType.X,
                op=mybir.AluOpType.add,
            )

            # ---- NCE: 16 row-chunks of M, exp+rowsum fused ----
            rowsums = wp.tile([128, 16], F32)
            escr = wp.tile([128, S], mybir.dt.bfloat16)
            for i in range(16):
                psM = pp.tile([128, S], F32, tag="ps")
                for j in range(4):
                    nc.tensor.matmul(
                        out=psM[:, ts(j, 512)],
                        lhsT=fqb[:, ts(i, 128)],
                        rhs=fkb[:, ts(j, 512)],
                        start=True,
                        stop=True,
                    )
                nc.scalar.activation(
                    escr[:], psM[:], AF.Exp,
                    bias=ebias[:, 0:1], scale=1.0 / TAU,
                    accum_out=rowsums[:, i:i + 1],
                )

            logt = wp.tile([128, 16], F32)
            nc.scalar.activation(logt[:], rowsums[:], AF.Ln)
            lred = wp.tile([128, 1], F32)
            nc.vector.tensor_reduce(
                lred[:], logt[:], axis=mybir.AxisListType.X,
                op=mybir.AluOpType.add,
            )
            psS = pp.tile([1, 1], F32, tag="ps")
            nc.tensor.matmul(
                out=psS[:], lhsT=ones128[:], rhs=lred[:], start=True, stop=True
            )
            nc.vector.tensor_copy(out_sb[0:1, 0:1], psS[:])
            nc.sync.dma_start(out_d, out_sb[:])


def _host_prep(f_q, f_k, W0, b0, W1, b1, c_ids, n_ids):
    """Build the per-core input maps (host-side sharding + constant prep)."""
    f_q = np.asarray(f_q, dtype=np.float32).reshape(B, C, HW)
    f_k = np.asarray(f_k, dtype=np.float32).reshape(B, C, HW)
    W0 = np.asarray(W0, dtype=np.float32)
    b0 = np.asarray(b0, dtype=np.float32)
    W1 = np.asarray(W1, dtype=np.float32)
    b1 = np.asarray(b1, dtype=np.float32)
    c_ids = np.asarray(c_ids).astype(np.int64)
    n_ids = np.asarray(n_ids).astype(np.int64)

    import ml_dtypes
    bf = ml_dtypes.bfloat16
    wblk = np.zeros((128, 128), np.float32)
    wblk[0:64, 0:64] = W0
    wblk[64:128, 64:128] = W0
    wblk = wblk.astype(bf)
    w1q = np.zeros((128, 16), np.float32)
    w1q[0:64, :] = W1
    w1q = w1q.astype(bf)
    w1k = np.zeros((128, 16), np.float32)
    w1k[64:128, :] = W1
    w1k = w1k.astype(bf)
    b0b = np.concatenate([b0, b0]).reshape(128, 1).astype(np.float32)
    b1c = b1.reshape(16, 1).astype(np.float32)

    def wrap_idx(ids):
        # w[p, j] = scratch row of sample s = j*128 + p; scratch rows are
        # permuted pair ids (see kernel comment): row(pid) =
        # (pid//1024)*1024 + (pid%128)*8 + (pid//128)%8
        pid = (ids >> 1).astype(np.int64)
        row = (pid // 1024) * 1024 + (pid % 128) * 8 + (pid // 128) % 8
        return row.astype(np.int32).reshape(NJ, 128).T.copy()

    def parity_mask(ids):
        return (ids & 1).astype(np.int8).reshape(NJ, 128).T.copy()

    common = {
        "wblk": wblk, "w1q": w1q, "w1k": w1k, "b0b": b0b, "b1c": b1c,
        "ones16": np.ones((16, 16), np.float32),
        "ones128": np.ones((128, 1), np.float32),
        "ident": np.eye(128, dtype=np.float16),
        "idxc": wrap_idx(c_ids), "idxn": wrap_idx(n_ids),
        "maskc": parity_mask(c_ids), "maskn": parity_mask(n_ids),
    }
    in_maps = []
    for b in range(B):
        m = dict(common)
        m["fs"] = np.concatenate([f_q[b], f_k[b]], axis=0)  # [128, HW]
        in_maps.append(m)
    return in_maps


def _finish(results):
    total = 0.0
    for r in results:
        o = np.asarray(r["out"], dtype=np.float64).reshape(2)
        total += S / TAU + o[0] - o[1] / TAU
    return np.float32(total / (B * S))


def kernel(**inputs) -> np.ndarray:
    nc = _build()
    in_maps = _host_prep(
        inputs["f_q"], inputs["f_k"], inputs["W0"], inputs["b0"],
        inputs["W1"], inputs["b1"], inputs["c_ids"], inputs["n_ids"],
    )
    res = bass_utils.run_bass_kernel_spmd(
        nc, in_maps, core_ids=list(range(NCORES))
    )
    return _finish(res.results)


# revision 35
# speedup vs baseline: 1.1659x; 1.1659x over previous
"""Trainium2 Bass kernel for nn_CCPL_14216341750304 (CCPL / PatchNCE loss).

Math (per batch b, one per NeuronCore, 8 cores):
    Z_m = f_m[b].T @ W0  for m in {q, k}            # [HW, 64], fused into PE transpose
    g_c = Z[c_ids], g_n = Z[n_ids]                  # runtime gather
    H   = relu(g_c - g_n + b0); E = H @ W1 + b1     # MLP
    F   = E / (||E||_2 + eps)                       # L2 norm over 16 ch
    M   = Fq.T @ Fk   [S, S]                        # cosine sims, |M| <= 1
    loss_row s = 1/tau + log(sum_t exp((M[s,t]-1)/tau)) - M[s,s]/tau
l_pos is exactly diag(M); masking the diag with -inf and concatenating
l_pos yields the same logsumexp multiset as the unmasked row.  |M|<=1
lets a constant shift of 1 replace the row-max (no overflow, no masking).

Implementation per core:
  Phase T: stream f (q||k stacked on 128 partitions) from HBM; one PE
    matmul per 128 pixels: lhsT = f-chunk [128ch, 128px], rhs =
    blockdiag(W0, W0) -> PSUM [128px, 128ch] — transpose + W0 in one op.
    Pixels pair-interleaved so a pixel PAIR forms one contiguous 512B
    row [even: q64,k64 | odd: q64,k64] fp16 of a DRAM scratch
    [32768 pairs, 256].
  Gather: 16+16 indirect DMAs (stock InstDMACopy, one row per partition,
    idx = pair id int32) -> [128, 256] fp16 tiles; parity resolved with
    copy_predicated; diff; PE transpose back to [128ch, S].
  MLP/normalize on [128, S]/[16, S]; NCE via 16 M-chunks of [128, 2048]
  PSUM, exp+rowsum fused on ScalarE (accum_out).
  Output [1, 2] per core: [sum_s log(rowsum_s), sum_s l_pos_s].
Host: loss = sum_cores(S/tau + o0 - o1/tau) / (8*S).
"""

import numpy as np

import concourse.bacc as bacc
import concourse.bass as bass
import concourse.mybir as mybir
import concourse.tile as tile
from concourse import bass_utils
from concourse.bass import ds, ts

F32 = mybir.dt.float32
F16 = mybir.dt.float16
BF16 = mybir.dt.bfloat16
I32 = mybir.dt.int32
I8 = mybir.dt.int8

B, C, H, W = 8, 64, 256, 256
HW = H * W                 # 65536
S = 2048                   # samples per batch (8*256)
NJ = S // 128              # 16 gather blocks per id set
TAU = 0.07
EPS = 1e-7
NCORES = 8
CHUNK = 4096               # pixels per DMA load chunk
NCHUNK = HW // CHUNK       # 16
PSCHUNK = 2048             # pixels per PSUM tile (8 superblocks of 256)
NPT = HW // PSCHUNK        # 32 psum tiles
EXPBIAS = -1.0 / TAU       # exp((M-1)/tau) = exp(M*(1/tau) + (-1/tau))

_CACHE = {}


def _build(n_bodies=1, stop_after=None, loop_n=0):
    """Build + compile the per-core Bass program (cached).

    n_bodies > 1 emits the whole kernel body multiple times back-to-back —
    used by the test harness to difference out fixed call overheads.
    stop_after in {"load", "T", "gather", "mlp"} truncates the body (for
    phase attribution in perf experiments).
    loop_n > 0 wraps the body in a device-side For loop of that many
    iterations (perf amplification).
    """
    key = f"nc{n_bodies}_{stop_after}_{loop_n}"
    if key in _CACHE:
        return _CACHE[key]

    nc = bacc.Bacc("TRN2", target_bir_lowering=False, debug=False)

    def dram_in(name, shape, dt):
        return nc.dram_tensor(name, shape, dt, kind="ExternalInput").ap()

    fs_d = dram_in("fs", [128, HW], F32)        # rows 0-63 fq ch, 64-127 fk ch
    wblk_d = dram_in("wblk", [128, 128], BF16)  # blockdiag(W0, W0)
    w1q_d = dram_in("w1q", [128, 16], BF16)     # W1 rows 0-63, zeros below
    w1k_d = dram_in("w1k", [128, 16], BF16)     # zeros, W1 rows 64-127
    b0b_d = dram_in("b0b", [128, 1], F32)       # [b0; b0]
    b1c_d = dram_in("b1c", [16, 1], F32)
    ones16_d = dram_in("ones16", [16, 16], F32)
    ones128_d = dram_in("ones128", [128, 1], F32)
    ident_d = dram_in("ident", [128, 128], F16)  # fp16 identity for PE transpose
    idxc_d = dram_in("idxc", [128, NJ], I32)    # pair ids, w[p,j] = pid[j*128+p]
    idxn_d = dram_in("idxn", [128, NJ], I32)
    maskc_d = dram_in("maskc", [128, NJ], I8)   # odd-parity bit per sample
    maskn_d = dram_in("maskn", [128, NJ], I8)
    out_d = nc.dram_tensor("out", [1, 2], F32, kind="ExternalOutput").ap()
    scr_d = nc.dram_tensor("zscr", [HW // 2, 256], F16, kind="Internal").ap()

    AF = mybir.ActivationFunctionType

    with tile.TileContext(nc) as tc:
        if loop_n:
            with tc.For_i(0, loop_n, 1):
                _emit_body(nc, tc, 0, AF, fs_d, wblk_d, w1q_d, w1k_d,
                           b0b_d, b1c_d, ones16_d, ones128_d, ident_d,
                           idxc_d, idxn_d, maskc_d, maskn_d, out_d, scr_d,
                           stop_after)
        else:
            for _body_i in range(n_bodies):
                _emit_body(nc, tc, _body_i, AF, fs_d, wblk_d, w1q_d, w1k_d,
                           b0b_d, b1c_d, ones16_d, ones128_d, ident_d,
                           idxc_d, idxn_d, maskc_d, maskn_d, out_d, scr_d,
                           stop_after)

    nc.compile()
    _CACHE[key] = nc
    return nc


def _emit_body(nc, tc, uid, AF, fs_d, wblk_d, w1q_d, w1k_d, b0b_d, b1c_d,
               ones16_d, ones128_d, ident_d, idxc_d, idxn_d, maskc_d,
               maskn_d, out_d, scr_d, stop_after=None):
        with (
            tc.tile_pool(name=f"const{uid}", bufs=1) as cp,
            tc.tile_pool(name=f"work{uid}", bufs=1) as wp,
            tc.tile_pool(name=f"psum{uid}", bufs=2,
                         space=bass.MemorySpace.PSUM) as pp,
        ):
            # ---- constants ----
            wblk = cp.tile([128, 128], BF16)
            nc.sync.dma_start(wblk[:], wblk_d)
            w1q = cp.tile([128, 16], BF16)
            nc.sync.dma_start(w1q[:], w1q_d)
            w1k = cp.tile([128, 16], BF16)
            nc.sync.dma_start(w1k[:], w1k_d)
            b0b = cp.tile([128, 1], F32)
            nc.sync.dma_start(b0b[:], b0b_d)
            b1c = cp.tile([16, 1], F32)
            nc.sync.dma_start(b1c[:], b1c_d)
            ones16 = cp.tile([16, 16], F32)
            nc.sync.dma_start(ones16[:], ones16_d)
            ones128 = cp.tile([128, 1], F32)
            nc.sync.dma_start(ones128[:], ones128_d)
            ident = cp.tile([128, 128], F16)
            nc.sync.dma_start(ident[:], ident_d)
            idxc = cp.tile([128, NJ], I32)
            nc.sync.dma_start(idxc[:], idxc_d)
            idxn = cp.tile([128, NJ], I32)
            nc.sync.dma_start(idxn[:], idxn_d)
            maskc = cp.tile([128, NJ], I8)
            nc.sync.dma_start(maskc[:], maskc_d)
            maskn = cp.tile([128, NJ], I8)
            nc.sync.dma_start(maskn[:], maskn_d)
            ebias = cp.tile([128, 1], F32)
            nc.gpsimd.memset(ebias[:], EXPBIAS)

            # ---- Phase T: transpose+W0 -> fp16 pair-slot scratch in DRAM ----
            # scratch row layout: [even px: q0..63,k0..63 | odd px: ...] fp16
            # = 512B. Row order is PERMUTED so each staging partition's 8
            # ranks are contiguous in DRAM (4KB per descriptor instead of
            # 512B): row(pid) = (pid//1024)*1024 + (pid%128)*8 + (pid//128)%8.
            # The host folds this permutation into the gather indices.
            scr_v = scr_d.rearrange("(pt p rl) e -> pt p rl e", p=128, rl=8)
            with tc.tile_pool(name=f"fst{uid}", bufs=3) as fsp:
                for ci in range(NCHUNK):
                    # SWDGE cast-DMA: f32 HBM -> bf16 SBUF at line rate
                    fstg = fsp.tile([128, CHUNK], BF16, tag="fst")
                    nc.gpsimd.dma_start(fstg[:], fs_d[:, ts(ci, CHUNK)])
                    if stop_after == "load":
                        continue
                    # free idx = h*2048 + sb*256 + q*2 + t
                    fsr = fstg[:].rearrange(
                        "p (h sb q t) -> p h sb t q", h=CHUNK // PSCHUNK,
                        sb=8, t=2
                    )
                    for hh in range(CHUNK // PSCHUNK):
                        ps = pp.tile([128, PSCHUNK], F32, tag="ps")
                        for sb in range(8):
                            for t in range(2):
                                nc.tensor.matmul(
                                    out=ps[:, ds(sb * 256 + t * 128, 128)],
                                    lhsT=fsr[:, hh, sb, t, :],
                                    rhs=wblk[:],
                                    start=True,
                                    stop=True,
                                )
                        pt = ci * (CHUNK // PSCHUNK) + hh
                        stg = fsp.tile([128, PSCHUNK], F16, tag="stg")
                        if pt % 2 == 0:
                            nc.vector.tensor_copy(stg[:], ps[:])
                        else:
                            nc.scalar.copy(stg[:], ps[:])
                        nc.sync.dma_start(
                            scr_v[pt],
                            stg[:].rearrange("p (rl e) -> p rl e", rl=8),
                        )

            if stop_after in ("load", "T"):
                dummy = wp.tile([1, 2], F32)
                nc.gpsimd.memset(dummy[:], 0.0)
                nc.sync.dma_start(out_d, dummy[:])
                return

            # ---- gathers: 16 blocks x [128 rows, 256] per id set ----
            gcs, gns = [], []
            for j in range(NJ):
                g = wp.tile([128, 256], F16, tag=f"gc{j}")
                nc.gpsimd.indirect_dma_start(
                    out=g[:], out_offset=None, in_=scr_d,
                    in_offset=bass.IndirectOffsetOnAxis(ap=idxc[:, j:j + 1], axis=0),
                )
                gcs.append(g)
                g = wp.tile([128, 256], F16, tag=f"gn{j}")
                nc.gpsimd.indirect_dma_start(
                    out=g[:], out_offset=None, in_=scr_d,
                    in_offset=bass.IndirectOffsetOnAxis(ap=idxn[:, j:j + 1], axis=0),
                )
                gns.append(g)

            if stop_after == "gather":
                dummy = wp.tile([1, 2], F32)
                nc.vector.tensor_copy(dummy[:], gcs[0][0:1, 0:2])
                nc.sync.dma_start(out_d, dummy[:])
                return

            # ---- parity select + diff (s-rows orientation) ----
            djall = wp.tile([128, S], F16)
            for j in range(NJ):
                mc = maskc[:, j:j + 1].to_broadcast([128, 128])
                mn = maskn[:, j:j + 1].to_broadcast([128, 128])
                nc.vector.copy_predicated(gcs[j][:, 0:128], mc, gcs[j][:, 128:256])
                nc.vector.copy_predicated(gns[j][:, 0:128], mn, gns[j][:, 128:256])
                nc.vector.tensor_sub(
                    djall[:, ts(j, 128)], gcs[j][:, 0:128], gns[j][:, 0:128]
                )

            # ---- transpose diff blocks back to [128ch, S] ----
            hin = wp.tile([128, S], F16)
            for j in range(NJ):
                pst = pp.tile([128, 128], F16, tag="ps")
                nc.tensor.transpose(
                    out=pst[:], in_=djall[:, ts(j, 128)], identity=ident[:]
                )
                if j % 2 == 0:
                    nc.vector.tensor_copy(hin[:, ts(j, 128)], pst[:])
                else:
                    nc.scalar.copy(hin[:, ts(j, 128)], pst[:])

            hid = wp.tile([128, S], BF16)
            nc.scalar.activation(hid[:], hin[:], AF.Relu, bias=b0b[:, 0:1])

            # ---- MLP layer 2 + L2 normalize ----
            fn = []
            for mi, w1t in enumerate((w1q, w1k)):
                psE = pp.tile([16, S], F32, tag="ps")
                for j in range(4):
                    nc.tensor.matmul(
                        out=psE[:, ts(j, 512)],
                        lhsT=w1t[:],
                        rhs=hid[:, ts(j, 512)],
                        start=True,
                        stop=True,
                    )
                emb = wp.tile([16, S], F32, tag="emb")
                nc.scalar.activation(emb[:], psE[:], AF.Identity, bias=b1c[:, 0:1])
                sq = wp.tile([16, S], F32, tag="sq")
                nc.vector.tensor_mul(sq[:], emb[:], emb[:])
                psN = pp.tile([16, S], F32, tag="ps")
                for j in range(4):
                    nc.tensor.matmul(
                        out=psN[:, ts(j, 512)],
                        lhsT=ones16[:],
                        rhs=sq[:, ts(j, 512)],
                        start=True,
                        stop=True,
                    )
                nrm = wp.tile([16, S], F32, tag="nrm")
                nc.scalar.activation(nrm[:], psN[:], AF.Sqrt)
                nrme = wp.tile([16, S], F32, tag="nrme")
                nc.vector.tensor_scalar_add(nrme[:], nrm[:], EPS)
                inv = wp.tile([16, S], F32, tag="inv")
                nc.vector.reciprocal_approx_fast(inv[:], nrme[:])
                f_n = wp.tile([16, S], F32, tag=f"fn{mi}")
                nc.vector.tensor_mul(f_n[:], emb[:], inv[:])
                fn.append(f_n)
            fqn, fkn = fn

            out_sb = wp.tile([1, 2], F32)

            if stop_after == "mlp":
                nc.vector.tensor_copy(out_sb[:], fqn[0:1, 0:2])
                nc.sync.dma_start(out_d, out_sb[:])
                return

            # bf16 copies for the NCE matmuls (PE runs bf16 at full rate)
            fqb = wp.tile([16, S], BF16)
            nc.vector.tensor_copy(fqb[:], fqn[:])
            fkb = wp.tile([16, S], BF16)
            nc.scalar.copy(fkb[:], fkn[:])

            # ---- l_pos = diag(M) = sum_c fqn*fkn ----
            pprod = wp.tile([16, S], F32, tag="sq")
            nc.vector.tensor_mul(pprod[:], fqn[:], fkn[:])
            psL = pp.tile([1, S], F32, tag="ps")
            for j in range(4):
                nc.tensor.matmul(
                    out=psL[:, ts(j, 512)],
                    lhsT=ones16[:, 0:1],
                    rhs=pprod[:, ts(j, 512)],
                    start=True,
                    stop=True,
                )
            nc.vector.tensor_reduce(
                out_sb[0:1, 1:2], psL[:], axis=mybir.AxisListType.X,
                op=mybir.AluOpType.add,
            )

            # ---- NCE: 16 row-chunks of M, exp+rowsum fused ----
            rowsums = wp.tile([128, 16], F32)
            escr = wp.tile([128, S], mybir.dt.bfloat16)
            for i in range(16):
                psM = pp.tile([128, S], F32, tag="ps")
                for j in range(4):
                    nc.tensor.matmul(
                        out=psM[:, ts(j, 512)],
                        lhsT=fqb[:, ts(i, 128)],
                        rhs=fkb[:, ts(j, 512)],
                        start=True,
                        stop=True,
                    )
                if i % 8 in (0, 3, 6):  # 6 direct, 10 offloaded: ACT~DVE balanced
                    # direct: ACT reads PSUM f32 (1x)
                    nc.scalar.activation(
                        escr[:], psM[:], AF.Exp,
                        bias=ebias[:, 0:1], scale=1.0 / TAU,
                        accum_out=rowsums[:, i:i + 1],
                    )
                else:
                    # offload PSUM read to DVE; ACT exp runs 2x from bf16 SBUF
                    msb = wp.tile([128, S], mybir.dt.bfloat16, tag="msb")
                    nc.vector.tensor_copy(msb[:], psM[:])
                    nc.scalar.activation(
                        escr[:], msb[:], AF.Exp,
                        bias=ebias[:, 0:1], scale=1.0 / TAU,
                        accum_out=rowsums[:, i:i + 1],
                    )

            logt = wp.tile([128, 16], F32)
            nc.scalar.activation(logt[:], rowsums[:], AF.Ln)
            lred = wp.tile([128, 1], F32)
            nc.vector.tensor_reduce(
                lred[:], logt[:], axis=mybir.AxisListType.X,
                op=mybir.AluOpType.add,
            )
            psS = pp.tile([1, 1], F32, tag="ps")
            nc.tensor.matmul(
                out=psS[:], lhsT=ones128[:], rhs=lred[:], start=True, stop=True
            )
            nc.vector.tensor_copy(out_sb[0:1, 0:1], psS[:])
            nc.sync.dma_start(out_d, out_sb[:])


def _host_prep(f_q, f_k, W0, b0, W1, b1, c_ids, n_ids):
    """Build the per-core input maps (host-side sharding + constant prep)."""
    f_q = np.asarray(f_q, dtype=np.float32).reshape(B, C, HW)
    f_k = np.asarray(f_k, dtype=np.float32).reshape(B, C, HW)
    W0 = np.asarray(W0, dtype=np.float32)
    b0 = np.asarray(b0, dtype=np.float32)
    W1 = np.asarray(W1, dtype=np.float32)
    b1 = np.asarray(b1, dtype=np.float32)
    c_ids = np.asarray(c_ids).astype(np.int64)
    n_ids = np.asarray(n_ids).astype(np.int64)

    import ml_dtypes
    bf = ml_dtypes.bfloat16
    wblk = np.zeros((128, 128), np.float32)
    wblk[0:64, 0:64] = W0
    wblk[64:128, 64:128] = W0
    wblk = wblk.astype(bf)
    w1q = np.zeros((128, 16), np.float32)
    w1q[0:64, :] = W1
    w1q = w1q.astype(bf)
    w1k = np.zeros((128, 16), np.float32)
    w1k[64:128, :] = W1
    w1k = w1k.astype(bf)
    b0b = np.concatenate([b0, b0]).reshape(128, 1).astype(np.float32)
    b1c = b1.reshape(16, 1).astype(np.float32)

    def wrap_idx(ids):
        # w[p, j] = scratch row of sample s = j*128 + p; scratch rows are
        # permuted pair ids (see kernel comment): row(pid) =
        # (pid//1024)*1024 + (pid%128)*8 + (pid//128)%8
        pid = (ids >> 1).astype(np.int64)
        row = (pid // 1024) * 1024 + (pid % 128) * 8 + (pid // 128) % 8
        return row.astype(np.int32).reshape(NJ, 128).T.copy()

    def parity_mask(ids):
        return (ids & 1).astype(np.int8).reshape(NJ, 128).T.copy()

    common = {
        "wblk": wblk, "w1q": w1q, "w1k": w1k, "b0b": b0b, "b1c": b1c,
        "ones16": np.ones((16, 16), np.float32),
        "ones128": np.ones((128, 1), np.float32),
        "ident": np.eye(128, dtype=np.float16),
        "idxc": wrap_idx(c_ids), "idxn": wrap_idx(n_ids),
        "maskc": parity_mask(c_ids), "maskn": parity_mask(n_ids),
    }
    in_maps = []
    for b in range(B):
        m = dict(common)
        m["fs"] = np.concatenate([f_q[b], f_k[b]], axis=0)  # [128, HW]
        in_maps.append(m)
    return in_maps


def _finish(results):
    total = 0.0
    for r in results:
        o = np.asarray(r["out"], dtype=np.float64).reshape(2)
        total += S / TAU + o[0] - o[1] / TAU
    return np.float32(total / (B * S))


def kernel(**inputs) -> np.ndarray:
    nc = _build()
    in_maps = _host_prep(
        inputs["f_q"], inputs["f_k"], inputs["W0"], inputs["b0"],
        inputs["W1"], inputs["b1"], inputs["c_ids"], inputs["n_ids"],
    )
    res = bass_utils.run_bass_kernel_spmd(
        nc, in_maps, core_ids=list(range(NCORES))
    )
    return _finish(res.results)
